# revision 25
# baseline (speedup 1.0000x reference)
"""GAT layer kernel for Trainium2, 8-core data-parallel over batch.

Math (per batch b, head h):
    h = x @ W                              [N, H*HD]
    s_i = <h_i, a_src[h]>,  t_j = <h_j, a_dst[h]>
    A[j, i] = exp(leakyrelu(s_i + t_j, 0.2))
    out[i]  = (sum_j A[j, i] * h_j) / (sum_j A[j, i])

Softmax over j is invariant to any per-column (per-i) scaling, so scale
column i by e^{-s_i}:
    Ā[j, i] = e^{-s_i} A[j, i] = max(e^{t_j}, e^{0.2 t_j} * e^{-0.8 s_i})
(exact: for s+t>=0 the left branch wins and equals e^{s+t-s}; below, the
right branch.) The left branch has no i-dependence, so each [128, N]
attention tile is ONE DVE tensor_scalar op with two per-partition scalars:
    a = (es08_bcast * etc02_col) max etc_col
in bf16 (2x DVE rate). The only broadcast tensor is es08[h] = e^{-0.8 s},
one per head, built by a rank-1-weight PE matmul (each weight column =
W @ a_src[h], via a stride-0 AP) into PSUM and an ACT exp into SBUF — no
DRAM broadcast round-trips.

t is produced directly in column form (t[j] on partitions) by
matmul(xT_tile, W @ a_dst) per node tile, skipping any transpose hop.

All PE inputs are f16 (full 1 col/cycle rate, half the DMA bytes of
f32r, ~0.05% rounding — well under the bf16 noise of the A tiles).

Aggregation: out^T[(h,d)|Z, i] accumulated in PSUM with a [h_node | ones]
weight block (Z row = column sums of Ā). Per head, the raw [33, N]
accumulator (numerator rows + Z row) is copied to SBUF by the otherwise
idle ACT engine and DMA'd out; the division by Z, the transpose back to
node-major, and the f32 cast happen on the host during unsharding.
"""

import numpy as np

B, N, IN_F, OUT_F, H = 8, 1024, 128, 128, 4
HD = OUT_F // H  # 32
NEG = 0.2
N_CORES = 8
NT = N // 128  # 8 node tiles

_CACHE = {}


def _build_nc():
    import concourse.bacc as bacc
    import concourse.tile as tile
    from concourse import mybir

    f32 = mybir.dt.float32
    f16 = mybir.dt.float16
    bf16 = mybir.dt.bfloat16
    AF = mybir.ActivationFunctionType
    ALU = mybir.AluOpType

    nc = bacc.Bacc("TRN2", target_bir_lowering=False, debug=False,
                   num_devices=N_CORES)

    # single contiguous input block [Wa | W | xT] so the load is few fat
    # descriptors (2320 B/row) instead of many thin ones
    CW = 2 * H + OUT_F + N
    inp = nc.declare_dram_parameter("inp", [IN_F, CW], f16, isOutput=False)
    onum = nc.declare_dram_parameter("onum", [H * 33, N], f32, isOutput=True)

    with tile.TileContext(nc) as tc:
      with (
        tc.tile_pool(name="const", bufs=1) as cpool,
        tc.tile_pool(name="atile", bufs=8) as apool,
        tc.tile_pool(name="otile", bufs=2) as opool,
      ):
        # ---- load input. The gpsimd software DGE spreads descriptors
        # across all 16 DMA engines; the HWDGE queues (sync/scalar)
        # serialize ~90ns/descriptor on one engine — 8x slower here.
        inp_sb = cpool.tile([IN_F, CW], f16, tag="inp")
        nc.gpsimd.dma_start(out=inp_sb, in_=inp[:])
        Wa_sb = inp_sb[:, 0:2 * H]
        W_sb = inp_sb[:, 2 * H:2 * H + OUT_F]
        xT_sb = inp_sb[:, 2 * H + OUT_F:CW]

        # prime the ACT exp table before it's on the critical path
        warm = cpool.tile([1, 8], f32, tag="warm")
        nc.scalar.activation(out=warm, in_=Wa_sb[0:1, 0:8], func=AF.Exp)

        # ---- t columns: tc_ps[j, 4*jt + h] = t_h[128*jt + j] ----
        with tc.tile_pool(name="ps_tc", bufs=1, space="PSUM") as pstc:
            tc_ps = pstc.tile([128, H * NT], f32, tag="tc")
            for jt in range(NT):
                nc.tensor.matmul(tc_ps[:, H * jt:H * (jt + 1)],
                                 xT_sb[:, 128 * jt:128 * (jt + 1)],
                                 Wa_sb[:, H:2 * H], start=True, stop=True)
            etc = cpool.tile([128, H * NT], f32, tag="etc")
            nc.scalar.activation(out=etc, in_=tc_ps, func=AF.Exp)
            etc02 = cpool.tile([128, H * NT], f32, tag="etc02")
            nc.scalar.activation(out=etc02, in_=tc_ps, func=AF.Exp, scale=NEG)

        # ---- es08_b[h][p, i] = e^{-0.8 s_h[i]}: rank-1-weight PE broadcast
        # matmul (every weight column = W a_src[h], stride-0 AP) + ACT exp.
        # bufs=1 serializes heads through one 2-bank slot; later heads are
        # emitted interleaved behind the main loop's matmuls.
        pssb = tc.tile_pool(name="ps_sb", bufs=1, space="PSUM")
        sbpool = pssb.__enter__()
        es08_b = {}

        def emit_sbcast(h):
            sb_ps = sbpool.tile([128, N], f32, tag="sb")
            wcol = Wa_sb[:, h:h + 1].to_broadcast([IN_F, 128])
            for c in range(2):
                nc.tensor.matmul(sb_ps[:, 512 * c:512 * (c + 1)], wcol,
                                 xT_sb[:, 512 * c:512 * (c + 1)],
                                 start=True, stop=True)
            eb = cpool.tile([128, N], bf16, tag=f"es08b{h}")
            nc.scalar.activation(out=eb, in_=sb_ps, func=AF.Exp, scale=-0.8)
            es08_b[h] = eb

        emit_sbcast(0)

        # ---- weight tiles: wt[:, 132jt+33h : +32] = h_node, col 32 = 1s.
        # hn matmuls are emitted interleaved into head 0's main loop (each
        # just ahead of the main matmul that consumes its wt block) so the
        # first main matmul issues right after sbcast h0.
        wt_all = cpool.tile([128, NT * 33 * H], bf16, tag="wt")
        wt_v = wt_all[:].rearrange("p (jt h c) -> p jt h c", h=H, c=33)
        nc.vector.memset(wt_v[:, :, :, 32:33], 1.0)
        wts = [wt_all[:, 132 * jt:132 * (jt + 1)] for jt in range(NT)]

        # numerator + Z rows out; the idle ACT engine does the PSUM read.
        # The last head splits the copy across ACT and DVE (both idle by
        # then) to shorten the trailing chain.
        def emit_out(h, oh, split=False):
            ocp = opool.tile([33, N], f32, tag="ocp")
            if split:
                nc.scalar.copy(out=ocp[0:HD, :], in_=oh[0:HD, :])
                nc.vector.tensor_copy(out=ocp[HD:33, :], in_=oh[HD:33, :])
            else:
                nc.scalar.copy(out=ocp, in_=oh)
            nc.gpsimd.dma_start(out=onum[33 * h:33 * (h + 1), :], in_=ocp)

        with tc.tile_pool(name="ps_main", bufs=2, space="PSUM") as psmain:
            # per-jt hn tiles (ring of 2) so hn matmul jt+1 never waits on
            # the wt copy of jt — a shared tile serialized PE behind DVE
            pshn = tc.tile_pool(name="ps_hn", bufs=2, space="PSUM")
            hnpool = pshn.__enter__()

            def emit_hn_mm(jt):
                hn_t = hnpool.tile([128, 128], f32, tag="hn")
                nc.tensor.matmul(hn_t,
                                 xT_sb[:, 128 * jt:128 * (jt + 1)], W_sb,
                                 start=True, stop=True)
                return hn_t

            def emit_hn_copy(jt, hn_t):
                nc.vector.tensor_copy(
                    out=wt_v[:, jt, :, 0:32],
                    in_=hn_t[:].rearrange("p (h c) -> p h c", c=32))

            hn_pend = emit_hn_mm(0)
            ohs = [None] * H
            for h in range(H):
                oh = psmain.tile([33, N], f32, tag="oh")
                ohs[h] = oh
                for jt in range(NT):
                    idx = H * jt + h
                    if h == 0:
                        emit_hn_copy(jt, hn_pend)
                    a_t = apool.tile([128, N], bf16, tag="at")
                    nc.vector.tensor_scalar(
                        out=a_t, in0=es08_b[h],
                        scalar1=etc02[:, idx:idx + 1],
                        scalar2=etc[:, idx:idx + 1],
                        op0=ALU.mult, op1=ALU.max)
                    if h == 0 and jt + 1 < NT:
                        hn_pend = emit_hn_mm(jt + 1)
                    if jt == 4 and h + 1 < H:
                        emit_sbcast(h + 1)
                    for c in range(2):
                        nc.tensor.matmul(
                            oh[:, 512 * c:512 * (c + 1)],
                            wts[jt][:, 33 * h:33 * (h + 1)],
                            a_t[:, 512 * c:512 * (c + 1)],
                            start=(jt == 0), stop=(jt == NT - 1))
                if h >= 1:
                    emit_out(h - 1, ohs[h - 1])
            emit_out(H - 1, ohs[H - 1], split=True)
            pshn.__exit__(None, None, None)
        pssb.__exit__(None, None, None)

    nc.compile()
    return nc


def _get_nc():
    if "nc" not in _CACHE:
        _CACHE["nc"] = _build_nc()
    return _CACHE["nc"]


def _prep_in_maps(x, W, a_src, a_dst):
    x = np.asarray(x, dtype=np.float32)
    W = np.asarray(W, dtype=np.float32)
    a_src = np.asarray(a_src, dtype=np.float32)
    a_dst = np.asarray(a_dst, dtype=np.float32)

    a_ext = np.zeros((OUT_F, 2 * H), np.float32)
    for h in range(H):
        a_ext[h * HD:(h + 1) * HD, h] = a_src[h]
        a_ext[h * HD:(h + 1) * HD, H + h] = a_dst[h]
    Wa = (W @ a_ext).astype(np.float16)
    W16 = W.astype(np.float16)

    return [
        {"inp": np.ascontiguousarray(np.concatenate(
            [Wa, W16, x[c].T.astype(np.float16)], axis=1))}
        for c in range(N_CORES)
    ]


def kernel(x, W, a_src, a_dst):
    from concourse.bass_utils import run_bass_kernel_spmd

    nc = _get_nc()
    in_maps = _prep_in_maps(x, W, a_src, a_dst)
    res = run_bass_kernel_spmd(nc, in_maps, core_ids=list(range(N_CORES)))
    out = np.empty((N_CORES, N, OUT_F), np.float32)
    for c in range(N_CORES):
        o = res.results[c]["onum"].reshape(H, 33, N)
        out[c] = (o[:, 0:HD, :] / o[:, HD:HD + 1, :]).transpose(2, 0, 1) \
            .reshape(N, OUT_F)
    return np.ascontiguousarray(out)


# revision 27
# speedup vs baseline: 1.0052x; 1.0052x over previous
"""GAT layer kernel for Trainium2, 8-core data-parallel over batch.

Math (per batch b, head h):
    h = x @ W                              [N, H*HD]
    s_i = <h_i, a_src[h]>,  t_j = <h_j, a_dst[h]>
    A[j, i] = exp(leakyrelu(s_i + t_j, 0.2))
    out[i]  = (sum_j A[j, i] * h_j) / (sum_j A[j, i])

Softmax over j is invariant to any per-column (per-i) scaling, so scale
column i by e^{-s_i}:
    Ā[j, i] = e^{-s_i} A[j, i] = max(e^{t_j}, e^{0.2 t_j} * e^{-0.8 s_i})
(exact: for s+t>=0 the left branch wins and equals e^{s+t-s}; below, the
right branch.) The left branch has no i-dependence, so each [128, N]
attention tile is ONE DVE tensor_scalar op with two per-partition scalars:
    a = (es08_bcast * etc02_col) max etc_col
in bf16 (2x DVE rate). The only broadcast tensor is es08[h] = e^{-0.8 s},
one per head: head 0 (latency-critical) via a rank-1-weight PE matmul
(host-precomputed weight block, every column = W a_src[0]) + ACT exp;
heads 1-3 via a [4, N] s-row matmul + exp + DRAM round-trip partition-
broadcast DMA on the gpsimd software DGE (which spreads descriptors
over all 16 DMA engines; the HWDGE queues serialize one engine).

t is produced directly in column form (t[j] on partitions) by
matmul(xT_tile, W @ a_dst) per node tile — no transpose hop.

All PE inputs are f16 (full 1 col/cycle rate, half the DMA bytes of
f32r, ~0.05% rounding — under the bf16 noise of the A tiles). The whole
input arrives as one contiguous [Wa | Wsb0 | W | xT] f16 block.

Aggregation: out^T[(h,d)|Z, i] accumulated in PSUM with a [h_node | 1s]
weight block (Z row = column sums of Ā). Per head, the raw [33, N]
accumulator (numerator + Z row) is copied to SBUF by the otherwise idle
ACT engine and DMA'd out raw; the division by Z, transpose to node-major
and f32 cast happen on the host during unsharding. The last head's copy
and DMA are split into column halves to shorten the trailing chain.
"""

import numpy as np

B, N, IN_F, OUT_F, H = 8, 1024, 128, 128, 4
HD = OUT_F // H  # 32
NEG = 0.2
N_CORES = 8
NT = N // 128  # 8 node tiles

_CACHE = {}


def _build_nc():
    import concourse.bacc as bacc
    import concourse.tile as tile
    from concourse import mybir

    f32 = mybir.dt.float32
    f16 = mybir.dt.float16
    bf16 = mybir.dt.bfloat16
    AF = mybir.ActivationFunctionType
    ALU = mybir.AluOpType

    nc = bacc.Bacc("TRN2", target_bir_lowering=False, debug=False,
                   num_devices=N_CORES)

    # contiguous input block: [Wa (2H) | Wsb0 (128) | W (128) | xT (1024)]
    CW = 2 * H + 128 + OUT_F + N
    inp = nc.declare_dram_parameter("inp", [IN_F, CW], f16, isOutput=False)
    onum = nc.declare_dram_parameter("onum", [H * 33, N], f32, isOutput=True)
    es_dram = nc.dram_tensor("es_scratch", [H, N], bf16)

    with tile.TileContext(nc) as tc:
      with (
        tc.tile_pool(name="const", bufs=1) as cpool,
        tc.tile_pool(name="atile", bufs=8) as apool,
        tc.tile_pool(name="otile", bufs=2) as opool,
      ):
        # ---- load input in two partition halves on the gpsimd SWDGE ----
        inp_sb = cpool.tile([IN_F, CW], f16, tag="inp")
        nc.gpsimd.dma_start(out=inp_sb[0:64, :], in_=inp[0:64, :])
        nc.gpsimd.dma_start(out=inp_sb[64:IN_F, :], in_=inp[64:IN_F, :])
        Wa_sb = inp_sb[:, 0:2 * H]
        Wsb_sb = inp_sb[:, 2 * H:2 * H + 128]
        W_sb = inp_sb[:, 2 * H + 128:2 * H + 128 + OUT_F]
        xT_sb = inp_sb[:, 2 * H + 128 + OUT_F:CW]

        # prime the ACT exp table before it's on the critical path
        warm = cpool.tile([1, 8], f32, tag="warm")
        nc.scalar.activation(out=warm, in_=inp_sb[0:1, 0:8], func=AF.Exp)

        es08_b = {}
        for h in range(H):
            eb = cpool.tile([128, N], bf16, tag=f"es08b{h}")
            es08_b[h] = eb

        with tc.tile_pool(name="ps_sb", bufs=1, space="PSUM") as sbpool:
            # head 0 broadcast via rank-1 weights, exp per column half so
            # the first a_t chunk starts as early as possible
            sb_ps = sbpool.tile([128, N], f32, tag="sb")
            for c in range(2):
                nc.tensor.matmul(sb_ps[:, 512 * c:512 * (c + 1)], Wsb_sb,
                                 xT_sb[:, 512 * c:512 * (c + 1)],
                                 start=True, stop=True)
                nc.scalar.activation(out=es08_b[0][:, 512 * c:512 * (c + 1)],
                                     in_=sb_ps[:, 512 * c:512 * (c + 1)],
                                     func=AF.Exp, scale=-0.8)
            # all four heads' s rows -> exp rows -> DRAM -> partition
            # broadcast for heads 1-3
            s_ps = sbpool.tile([H, N], f32, tag="srow")
            for c in range(2):
                nc.tensor.matmul(s_ps[:, 512 * c:512 * (c + 1)],
                                 Wa_sb[:, 0:H],
                                 xT_sb[:, 512 * c:512 * (c + 1)],
                                 start=True, stop=True)
            esr = cpool.tile([H, N], bf16, tag="esr")
            nc.scalar.activation(out=esr, in_=s_ps, func=AF.Exp, scale=-0.8)
            nc.gpsimd.dma_start(out=es_dram[:], in_=esr)
            for h in range(1, H):
                nc.gpsimd.dma_start(
                    out=es08_b[h],
                    in_=es_dram[h:h + 1, :].to_broadcast([128, N]))

        # ---- t columns: tc_ps[j, 4*jt + h] = t_h[128*jt + j] ----
        with tc.tile_pool(name="ps_tc", bufs=1, space="PSUM") as pstc:
            tc_ps = pstc.tile([128, H * NT], f32, tag="tc")
            for jt in range(NT):
                nc.tensor.matmul(tc_ps[:, H * jt:H * (jt + 1)],
                                 xT_sb[:, 128 * jt:128 * (jt + 1)],
                                 Wa_sb[:, H:2 * H], start=True, stop=True)
            etc = cpool.tile([128, H * NT], f32, tag="etc")
            nc.scalar.activation(out=etc, in_=tc_ps, func=AF.Exp)
            etc02 = cpool.tile([128, H * NT], f32, tag="etc02")
            nc.scalar.activation(out=etc02, in_=tc_ps, func=AF.Exp, scale=NEG)

        # ---- weight tiles: wt[:, 132jt+33h : +32] = h_node, col 32 = 1s ----
        wt_all = cpool.tile([128, NT * 33 * H], bf16, tag="wt")
        wt_v = wt_all[:].rearrange("p (jt h c) -> p jt h c", h=H, c=33)
        nc.vector.memset(wt_v[:, :, :, 32:33], 1.0)
        wts = [wt_all[:, 132 * jt:132 * (jt + 1)] for jt in range(NT)]

        # numerator + Z rows out; idle ACT engine reads PSUM. Last head is
        # split into column halves to shorten the trailing chain.
        def emit_out(h, oh, split=False):
            ocp = opool.tile([33, N], f32, tag="ocp")
            if split:
                for c in range(2):
                    nc.scalar.copy(out=ocp[:, 512 * c:512 * (c + 1)],
                                   in_=oh[:, 512 * c:512 * (c + 1)])
                    nc.gpsimd.dma_start(
                        out=onum[33 * h:33 * (h + 1), 512 * c:512 * (c + 1)],
                        in_=ocp[:, 512 * c:512 * (c + 1)])
            else:
                nc.scalar.copy(out=ocp, in_=oh)
                nc.gpsimd.dma_start(out=onum[33 * h:33 * (h + 1), :], in_=ocp)

        with tc.tile_pool(name="ps_main", bufs=3, space="PSUM") as psmain:
            # hn tiles: 4 node-tiles per 1-bank PSUM tile, one wt copy per
            # batch of 4 so PE never interlocks with DVE per tile
            pshn = tc.tile_pool(name="ps_hn", bufs=2, space="PSUM")
            hnpool = pshn.__enter__()

            def emit_hn_batch(g):
                hn_t = hnpool.tile([128, 512], f32, tag="hn")
                for q in range(4):
                    jt = 4 * g + q
                    nc.tensor.matmul(hn_t[:, 128 * q:128 * (q + 1)],
                                     xT_sb[:, 128 * jt:128 * (jt + 1)], W_sb,
                                     start=True, stop=True)
                nc.vector.tensor_copy(
                    out=wt_v[:, 4 * g:4 * (g + 1), :, 0:32],
                    in_=hn_t[:].rearrange("p (jt h c) -> p jt h c",
                                          h=H, c=32))

            emit_hn_batch(0)
            emit_hn_batch(1)
            ohs = [None] * H
            for h in range(H):
                oh = psmain.tile([33, N], f32, tag="oh")
                ohs[h] = oh
                for jt in range(NT):
                    idx = H * jt + h
                    a_t = apool.tile([128, N], bf16, tag="at")
                    if h == 0 and jt == 0:
                        for c in range(2):
                            nc.vector.tensor_scalar(
                                out=a_t[:, 512 * c:512 * (c + 1)],
                                in0=es08_b[h][:, 512 * c:512 * (c + 1)],
                                scalar1=etc02[:, idx:idx + 1],
                                scalar2=etc[:, idx:idx + 1],
                                op0=ALU.mult, op1=ALU.max)
                    else:
                        nc.vector.tensor_scalar(
                            out=a_t, in0=es08_b[h],
                            scalar1=etc02[:, idx:idx + 1],
                            scalar2=etc[:, idx:idx + 1],
                            op0=ALU.mult, op1=ALU.max)
                    for c in range(2):
                        nc.tensor.matmul(
                            oh[:, 512 * c:512 * (c + 1)],
                            wts[jt][:, 33 * h:33 * (h + 1)],
                            a_t[:, 512 * c:512 * (c + 1)],
                            start=(jt == 0), stop=(jt == NT - 1))
                if h >= 1:
                    emit_out(h - 1, ohs[h - 1])
            emit_out(H - 1, ohs[H - 1], split=True)
            pshn.__exit__(None, None, None)

    nc.compile()
    return nc


def _get_nc():
    if "nc" not in _CACHE:
        _CACHE["nc"] = _build_nc()
    return _CACHE["nc"]


def _prep_in_maps(x, W, a_src, a_dst):
    x = np.asarray(x, dtype=np.float32)
    W = np.asarray(W, dtype=np.float32)
    a_src = np.asarray(a_src, dtype=np.float32)
    a_dst = np.asarray(a_dst, dtype=np.float32)

    a_ext = np.zeros((OUT_F, 2 * H), np.float32)
    for h in range(H):
        a_ext[h * HD:(h + 1) * HD, h] = a_src[h]
        a_ext[h * HD:(h + 1) * HD, H + h] = a_dst[h]
    Wa = (W @ a_ext).astype(np.float16)
    Wsb0 = np.repeat(Wa[:, 0:1], 128, axis=1)
    W16 = W.astype(np.float16)

    return [
        {"inp": np.ascontiguousarray(np.concatenate(
            [Wa, Wsb0, W16, x[c].T.astype(np.float16)], axis=1))}
        for c in range(N_CORES)
    ]


def kernel(x, W, a_src, a_dst):
    from concourse.bass_utils import run_bass_kernel_spmd

    nc = _get_nc()
    in_maps = _prep_in_maps(x, W, a_src, a_dst)
    res = run_bass_kernel_spmd(nc, in_maps, core_ids=list(range(N_CORES)))
    out = np.empty((N_CORES, N, OUT_F), np.float32)
    for c in range(N_CORES):
        o = res.results[c]["onum"].reshape(H, 33, N)
        out[c] = (o[:, 0:HD, :] / o[:, HD:HD + 1, :]).transpose(2, 0, 1) \
            .reshape(N, OUT_F)
    return np.ascontiguousarray(out)


# revision 28
# speedup vs baseline: 1.0144x; 1.0091x over previous
"""GAT layer kernel for Trainium2, 8-core data-parallel over batch.

Math (per batch b, head h):
    h = x @ W                              [N, H*HD]
    s_i = <h_i, a_src[h]>,  t_j = <h_j, a_dst[h]>
    A[j, i] = exp(leakyrelu(s_i + t_j, 0.2))
    out[i]  = (sum_j A[j, i] * h_j) / (sum_j A[j, i])

Softmax over j is invariant to any per-column (per-i) scaling, so scale
column i by e^{-s_i}:
    Ā[j, i] = e^{-s_i} A[j, i] = max(e^{t_j}, e^{0.2 t_j} * e^{-0.8 s_i})
(exact: for s+t>=0 the left branch wins and equals e^{s+t-s}; below, the
right branch.) The left branch has no i-dependence, so each [128, N]
attention tile is ONE DVE tensor_scalar op with two per-partition scalars:
    a = (es08_bcast * etc02_col) max etc_col
in bf16 (2x DVE rate). The only broadcast tensor is es08[h] = e^{-0.8 s},
one per head: head 0 (latency-critical) via a rank-1-weight PE matmul
(host-precomputed weight block, every column = W a_src[0]) + ACT exp;
heads 1-3 via a [4, N] s-row matmul + exp + DRAM round-trip partition-
broadcast DMA on the gpsimd software DGE (which spreads descriptors
over all 16 DMA engines; the HWDGE queues serialize one engine).

t is produced directly in column form (t[j] on partitions) by
matmul(xT_tile, W @ a_dst) per node tile — no transpose hop.

All PE inputs are f16 (full 1 col/cycle rate, half the DMA bytes of
f32r, ~0.05% rounding — under the bf16 noise of the A tiles). The whole
input arrives as one contiguous [Wa | Wsb0 | W | xT] f16 block.

Aggregation: out^T[(h,d)|Z, i] accumulated in PSUM with a [h_node | 1s]
weight block (Z row = column sums of Ā). Per head, the raw [33, N]
accumulator (numerator + Z row) is copied to SBUF by the otherwise idle
ACT engine and DMA'd out raw; the division by Z, transpose to node-major
and f32 cast happen on the host during unsharding. The last head's copy
and DMA are split into column halves to shorten the trailing chain.
"""

import numpy as np

B, N, IN_F, OUT_F, H = 8, 1024, 128, 128, 4
HD = OUT_F // H  # 32
NEG = 0.2
N_CORES = 8
NT = N // 128  # 8 node tiles

_CACHE = {}


def _build_nc():
    import concourse.bacc as bacc
    import concourse.tile as tile
    from concourse import mybir

    f32 = mybir.dt.float32
    f16 = mybir.dt.float16
    bf16 = mybir.dt.bfloat16
    AF = mybir.ActivationFunctionType
    ALU = mybir.AluOpType

    nc = bacc.Bacc("TRN2", target_bir_lowering=False, debug=False,
                   num_devices=N_CORES)

    # contiguous input block: [Wa (2H) | Wsb0 (128) | W (128) | xT (1024)]
    CW = 2 * H + 128 + OUT_F + N
    inp = nc.declare_dram_parameter("inp", [IN_F, CW], f16, isOutput=False)
    onum = nc.declare_dram_parameter("onum", [H * 33, N], f32, isOutput=True)
    es_dram = nc.dram_tensor("es_scratch", [H, N], bf16)

    with tile.TileContext(nc) as tc:
      with (
        tc.tile_pool(name="const", bufs=1) as cpool,
        tc.tile_pool(name="atile", bufs=8) as apool,
        tc.tile_pool(name="otile", bufs=2) as opool,
      ):
        # ---- load input in two partition halves on the gpsimd SWDGE ----
        inp_sb = cpool.tile([IN_F, CW], f16, tag="inp")
        nc.gpsimd.dma_start(out=inp_sb[0:64, :], in_=inp[0:64, :])
        nc.gpsimd.dma_start(out=inp_sb[64:IN_F, :], in_=inp[64:IN_F, :])
        Wa_sb = inp_sb[:, 0:2 * H]
        Wsb_sb = inp_sb[:, 2 * H:2 * H + 128]
        W_sb = inp_sb[:, 2 * H + 128:2 * H + 128 + OUT_F]
        xT_sb = inp_sb[:, 2 * H + 128 + OUT_F:CW]

        # prime the ACT exp table before it's on the critical path
        warm = cpool.tile([1, 8], f32, tag="warm")
        nc.scalar.activation(out=warm, in_=inp_sb[0:1, 0:8], func=AF.Exp)

        es08_b = {}
        for h in range(H):
            eb = cpool.tile([128, N], bf16, tag=f"es08b{h}")
            es08_b[h] = eb

        # ---- t columns first (they unlock the whole DVE pipeline and cost
        # ~0.4us PE): tc_ps[j, 4*jt + h] = t_h[128*jt + j]. Own pool bank so
        # nothing serializes against the s-row chain below.
        pstc = tc.tile_pool(name="ps_tc", bufs=1, space="PSUM")
        tcpool = pstc.__enter__()
        tc_ps = tcpool.tile([128, H * NT], f32, tag="tc")
        for jt in range(NT):
            nc.tensor.matmul(tc_ps[:, H * jt:H * (jt + 1)],
                             xT_sb[:, 128 * jt:128 * (jt + 1)],
                             Wa_sb[:, H:2 * H], start=True, stop=True)
        etc = cpool.tile([128, H * NT], f32, tag="etc")
        nc.scalar.activation(out=etc, in_=tc_ps, func=AF.Exp)
        etc02 = cpool.tile([128, H * NT], f32, tag="etc02")
        nc.scalar.activation(out=etc02, in_=tc_ps, func=AF.Exp, scale=NEG)

        with tc.tile_pool(name="ps_sb", bufs=1, space="PSUM") as sbpool:
            # head 0 broadcast via rank-1 weights, exp per column half so
            # the first a_t chunk starts as early as possible
            sb_ps = sbpool.tile([128, N], f32, tag="sb")
            for c in range(2):
                nc.tensor.matmul(sb_ps[:, 512 * c:512 * (c + 1)], Wsb_sb,
                                 xT_sb[:, 512 * c:512 * (c + 1)],
                                 start=True, stop=True)
                nc.scalar.activation(out=es08_b[0][:, 512 * c:512 * (c + 1)],
                                     in_=sb_ps[:, 512 * c:512 * (c + 1)],
                                     func=AF.Exp, scale=-0.8)
            # all four heads' s rows -> exp rows -> DRAM -> partition
            # broadcast for heads 1-3
            s_ps = sbpool.tile([H, N], f32, tag="srow")
            for c in range(2):
                nc.tensor.matmul(s_ps[:, 512 * c:512 * (c + 1)],
                                 Wa_sb[:, 0:H],
                                 xT_sb[:, 512 * c:512 * (c + 1)],
                                 start=True, stop=True)
            esr = cpool.tile([H, N], bf16, tag="esr")
            nc.scalar.activation(out=esr, in_=s_ps, func=AF.Exp, scale=-0.8)
            nc.gpsimd.dma_start(out=es_dram[:], in_=esr)
            for h in range(1, H):
                nc.gpsimd.dma_start(
                    out=es08_b[h],
                    in_=es_dram[h:h + 1, :].to_broadcast([128, N]))
        pstc.__exit__(None, None, None)

        # ---- weight tiles: wt[:, 132jt+33h : +32] = h_node, col 32 = 1s ----
        wt_all = cpool.tile([128, NT * 33 * H], bf16, tag="wt")
        wt_v = wt_all[:].rearrange("p (jt h c) -> p jt h c", h=H, c=33)
        nc.vector.memset(wt_v[:, :, :, 32:33], 1.0)
        wts = [wt_all[:, 132 * jt:132 * (jt + 1)] for jt in range(NT)]

        # numerator + Z rows out; idle ACT engine reads PSUM. Last head is
        # split into column halves to shorten the trailing chain.
        def emit_out(h, oh, split=False):
            ocp = opool.tile([33, N], f32, tag="ocp")
            if split:
                for c in range(2):
                    nc.scalar.copy(out=ocp[:, 512 * c:512 * (c + 1)],
                                   in_=oh[:, 512 * c:512 * (c + 1)])
                    nc.gpsimd.dma_start(
                        out=onum[33 * h:33 * (h + 1), 512 * c:512 * (c + 1)],
                        in_=ocp[:, 512 * c:512 * (c + 1)])
            else:
                nc.scalar.copy(out=ocp, in_=oh)
                nc.gpsimd.dma_start(out=onum[33 * h:33 * (h + 1), :], in_=ocp)

        with tc.tile_pool(name="ps_main", bufs=3, space="PSUM") as psmain:
            # hn tiles: 4 node-tiles per 1-bank PSUM tile, one wt copy per
            # batch of 4 so PE never interlocks with DVE per tile
            pshn = tc.tile_pool(name="ps_hn", bufs=2, space="PSUM")
            hnpool = pshn.__enter__()

            def emit_hn_batch(g):
                hn_t = hnpool.tile([128, 512], f32, tag="hn")
                for q in range(4):
                    jt = 4 * g + q
                    nc.tensor.matmul(hn_t[:, 128 * q:128 * (q + 1)],
                                     xT_sb[:, 128 * jt:128 * (jt + 1)], W_sb,
                                     start=True, stop=True)
                nc.vector.tensor_copy(
                    out=wt_v[:, 4 * g:4 * (g + 1), :, 0:32],
                    in_=hn_t[:].rearrange("p (jt h c) -> p jt h c",
                                          h=H, c=32))

            emit_hn_batch(0)
            emit_hn_batch(1)
            ohs = [None] * H
            for h in range(H):
                oh = psmain.tile([33, N], f32, tag="oh")
                ohs[h] = oh
                for jt in range(NT):
                    idx = H * jt + h
                    a_t = apool.tile([128, N], bf16, tag="at")
                    if h == 0 and jt == 0:
                        for c in range(2):
                            nc.vector.tensor_scalar(
                                out=a_t[:, 512 * c:512 * (c + 1)],
                                in0=es08_b[h][:, 512 * c:512 * (c + 1)],
                                scalar1=etc02[:, idx:idx + 1],
                                scalar2=etc[:, idx:idx + 1],
                                op0=ALU.mult, op1=ALU.max)
                    else:
                        nc.vector.tensor_scalar(
                            out=a_t, in0=es08_b[h],
                            scalar1=etc02[:, idx:idx + 1],
                            scalar2=etc[:, idx:idx + 1],
                            op0=ALU.mult, op1=ALU.max)
                    for c in range(2):
                        nc.tensor.matmul(
                            oh[:, 512 * c:512 * (c + 1)],
                            wts[jt][:, 33 * h:33 * (h + 1)],
                            a_t[:, 512 * c:512 * (c + 1)],
                            start=(jt == 0), stop=(jt == NT - 1))
                if h >= 1:
                    emit_out(h - 1, ohs[h - 1])
            emit_out(H - 1, ohs[H - 1], split=True)
            pshn.__exit__(None, None, None)

    nc.compile()
    return nc


def _get_nc():
    if "nc" not in _CACHE:
        _CACHE["nc"] = _build_nc()
    return _CACHE["nc"]


def _prep_in_maps(x, W, a_src, a_dst):
    x = np.asarray(x, dtype=np.float32)
    W = np.asarray(W, dtype=np.float32)
    a_src = np.asarray(a_src, dtype=np.float32)
    a_dst = np.asarray(a_dst, dtype=np.float32)

    a_ext = np.zeros((OUT_F, 2 * H), np.float32)
    for h in range(H):
        a_ext[h * HD:(h + 1) * HD, h] = a_src[h]
        a_ext[h * HD:(h + 1) * HD, H + h] = a_dst[h]
    Wa = (W @ a_ext).astype(np.float16)
    Wsb0 = np.repeat(Wa[:, 0:1], 128, axis=1)
    W16 = W.astype(np.float16)

    return [
        {"inp": np.ascontiguousarray(np.concatenate(
            [Wa, Wsb0, W16, x[c].T.astype(np.float16)], axis=1))}
        for c in range(N_CORES)
    ]


def kernel(x, W, a_src, a_dst):
    from concourse.bass_utils import run_bass_kernel_spmd

    nc = _get_nc()
    in_maps = _prep_in_maps(x, W, a_src, a_dst)
    res = run_bass_kernel_spmd(nc, in_maps, core_ids=list(range(N_CORES)))
    out = np.empty((N_CORES, N, OUT_F), np.float32)
    for c in range(N_CORES):
        o = res.results[c]["onum"].reshape(H, 33, N)
        out[c] = (o[:, 0:HD, :] / o[:, HD:HD + 1, :]).transpose(2, 0, 1) \
            .reshape(N, OUT_F)
    return np.ascontiguousarray(out)
